# revision 12
# baseline (speedup 1.0000x reference)
"""Trainium2 Bass kernel for nn_CharRNN: bidirectional char-GRU + temporal max-pool.

Problem shapes (hardcoded): B=64, S=256, T=16, V=262, E=64, H=32.
16384 independent char sequences ("words") are sharded 8 ways (2048 words/core).

Design (v2, latency-oriented):
- The input-gate projections gi = emb[x] @ Wih.T (+ all foldable biases) take
  only 262 distinct values per gate; the host precomputes the projected table
  P[V, 3, 32] per direction and ships the per-step gathered gi tiles in bf16
  (12.6MB/core) instead of a one-hot (16.8MB/core) + on-device projection.
  This removes the embedding matmul prologue and all 6 per-step ih matmuls.
- Two independent software-pipelined chains (forward dir, backward dir), each
  with tiles [128 = 32 hidden dims x 4 word-groups, 512 words]. The two
  chains fill each other's latency bubbles on every engine.
- Per step and chain: PE does 3 hh matmuls (block-diag packed K=128) plus 2
  identity-matmul injections of gi_r|gi_z into PSUM; ACT does one fused
  sigmoid over [r|z] (2 PSUM banks) and one tanh; DVE does
  hn=(psumN+bhh_n)*r (scalar_tensor_tensor), npre=hn+gi_n, and the h' mix;
  GpSimd does the running max-pool.

Per-step math (PyTorch GRU cell, biases pre-folded into gi):
  r = sigmoid(gi_r + Whh_r h)          gi_r includes b_ih_r + b_hh_r
  z = sigmoid(gi_z + Whh_z h)          gi_z includes b_ih_z + b_hh_z
  n = tanh(gi_n + r*(Whh_n h + b_hh_n))   gi_n includes b_ih_n
  h' = n + z*(h - n);  ymax = max(ymax, h')
"""

import sys
import os

sys.path.insert(0, "/opt/trn_rl_repo")

import numpy as np

import concourse.bacc as bacc
import concourse.tile as tile
from concourse import mybir
from concourse.bass_utils import run_bass_kernel_spmd
from concourse.alu_op_type import AluOpType as Alu

B, S, T = 64, 256, 16
V, E, H = 262, 64, 32
NCORES = 8
WPC = 16384 // NCORES  # words per core = 2048
NG = 4                 # word groups per direction chain
GW = WPC // NG         # words per group = free width = 512

F32 = mybir.dt.float32
BF16 = mybir.dt.bfloat16
AF = mybir.ActivationFunctionType

DIRS = ("f", "b")

_CACHE = {}


def _build_program():
    nc = bacc.Bacc("TRN2", target_bir_lowering=False, debug=False, num_devices=NCORES)

    d_giRZ = {
        d: nc.dram_tensor(f"giRZ_{d}", [T, 128, 2 * GW], BF16, kind="ExternalInput").ap()
        for d in DIRS
    }
    d_giN = {
        d: nc.dram_tensor(f"giN_{d}", [T, 128, GW], BF16, kind="ExternalInput").ap()
        for d in DIRS
    }
    d_hh = {
        (d, g): nc.dram_tensor(f"hh{g}_{d}", [128, 128], BF16, kind="ExternalInput").ap()
        for d in DIRS
        for g in "RZN"
    }
    d_ident = nc.dram_tensor("ident", [128, 128], BF16, kind="ExternalInput").ap()
    d_bhh = {
        d: nc.dram_tensor(f"bhhN_{d}", [128, 1], F32, kind="ExternalInput").ap()
        for d in DIRS
    }
    d_out = {
        d: nc.dram_tensor(f"out_{d}", [128, GW], BF16, kind="ExternalOutput").ap()
        for d in DIRS
    }

    with tile.TileContext(nc) as tc:
        with (
            tc.tile_pool(name="consts", bufs=1) as consts,
            tc.tile_pool(name="gi", bufs=6) as gip,
            tc.tile_pool(name="state", bufs=1) as state,
            tc.tile_pool(name="work", bufs=2) as work,
            tc.tile_pool(name="psRZ", bufs=1, space="PSUM") as psRZ,
            tc.tile_pool(name="psN", bufs=1, space="PSUM") as psN,
        ):
            # ---- constants ----
            s_hh = {}
            for (d, g), dt_ in d_hh.items():
                s_hh[(d, g)] = consts.tile([128, 128], BF16, name=f"hh{g}{d}")
                nc.sync.dma_start(out=s_hh[(d, g)], in_=dt_)
            s_id = consts.tile([128, 128], BF16, name="ident")
            nc.sync.dma_start(out=s_id, in_=d_ident)
            s_bhh = {}
            for d in DIRS:
                s_bhh[d] = consts.tile([128, 1], F32, name=f"bhh{d}")
                nc.sync.dma_start(out=s_bhh[d], in_=d_bhh[d])

            # ---- state ----
            h = {}
            h0 = state.tile([128, 2 * GW], BF16, name="h0")
            nc.vector.memset(h0, 0.0)
            for i, d in enumerate(DIRS):
                h[d] = h0[:, i * GW:(i + 1) * GW]
            ymax = state.tile([128, 2 * GW], BF16, name="ymax")

            # ---- steps ----
            # gi_r|gi_z are injected into their PSUM banks EARLY (start=True
            # identity matmuls, off the critical path); the hh matmuls then
            # accumulate (start=False) once h' of the previous step lands, so
            # the sigmoid prefix is just one matmul deep.
            pRZ, pN = {}, {}

            def inject_rz(s):
                # DMA split across queues (sync+gpsimd; 4-way on the first
                # steps where the serial transfer would gate the pipeline)
                giRZ = {}
                R, Z = slice(0, GW), slice(GW, 2 * GW)
                for d in DIRS:
                    giRZ[d] = gip.tile([128, 2 * GW], BF16, tag=f"giRZ{d}", name=f"giRZ_{d}{s}")
                    if s < 2:
                        HGW = GW // 2
                        for k, eng in enumerate((nc.sync, nc.gpsimd, nc.scalar, nc.gpsimd)):
                            sl = slice(k * HGW, (k + 1) * HGW)
                            eng.dma_start(out=giRZ[d][:, sl], in_=d_giRZ[d][s, :, sl])
                    else:
                        nc.sync.dma_start(out=giRZ[d][:, R], in_=d_giRZ[d][s, :, R])
                        nc.gpsimd.dma_start(out=giRZ[d][:, Z], in_=d_giRZ[d][s, :, Z])
                last = s == 0
                for d in DIRS:
                    pRZ[d] = psRZ.tile([128, 2 * GW], F32, tag=f"rz{d}", name=f"pRZ_{d}{s}")
                    nc.tensor.matmul(pRZ[d][:, R], lhsT=s_id, rhs=giRZ[d][:, R], start=True, stop=last)
                    nc.tensor.matmul(pRZ[d][:, Z], lhsT=s_id, rhs=giRZ[d][:, Z], start=True, stop=last)

            giN = {}

            def fetch_gin(s):
                for d in DIRS:
                    giN[d] = gip.tile([128, GW], BF16, tag=f"giN{d}", name=f"giN_{d}{s}")
                    if s < 2:
                        HGW = GW // 2
                        nc.sync.dma_start(out=giN[d][:, 0:HGW], in_=d_giN[d][s, :, 0:HGW])
                        nc.gpsimd.dma_start(out=giN[d][:, HGW:GW], in_=d_giN[d][s, :, HGW:GW])
                    else:
                        eng = nc.gpsimd if s % 2 else nc.sync
                        eng.dma_start(out=giN[d], in_=d_giN[d][s])

            inject_rz(0)
            fetch_gin(0)

            for s in range(T):
                R, Z = slice(0, GW), slice(GW, 2 * GW)
                giN_s = dict(giN)
                # PE: hh accumulations (critical: R first), f then b.
                # h(0) == 0, so step 0 has no hh contributions at all.
                for d in DIRS:
                    if s > 0:
                        pN[d] = psN.tile([128, GW], F32, tag=f"n{d}", name=f"pN_{d}{s}")
                        nc.tensor.matmul(pRZ[d][:, R], lhsT=s_hh[(d, "R")], rhs=h[d], start=False, stop=True)
                        nc.tensor.matmul(pRZ[d][:, Z], lhsT=s_hh[(d, "Z")], rhs=h[d], start=False, stop=True)
                        nc.tensor.matmul(pN[d], lhsT=s_hh[(d, "N")], rhs=h[d], start=True, stop=True)

                pRZ_s = dict(pRZ)
                rz, npre, n = {}, {}, {}
                for d in DIRS:
                    # ACT: split sigmoids — r alone unblocks the N-path
                    rz[d] = work.tile([128, 2 * GW], BF16, tag=f"rzs{d}", name=f"rz_{d}{s}")
                    nc.scalar.activation(rz[d][:, R], pRZ_s[d][:, R], AF.Sigmoid)
                    nc.scalar.activation(rz[d][:, Z], pRZ_s[d][:, Z], AF.Sigmoid)
                    # DVE: hn = (psumN + bhh_n) * r ; npre = hn + gi_n
                    hn = work.tile([128, GW], BF16, tag=f"hn{d}", name=f"hn_{d}{s}")
                    if s > 0:
                        nc.vector.scalar_tensor_tensor(
                            out=hn, in0=pN[d], scalar=s_bhh[d], in1=rz[d][:, R],
                            op0=Alu.add, op1=Alu.mult,
                        )
                    else:  # psumN == 0: hn = bhh_n * r
                        nc.vector.tensor_scalar(
                            out=hn, in0=rz[d][:, R], scalar1=s_bhh[d],
                            scalar2=None, op0=Alu.mult,
                        )
                    npre[d] = work.tile([128, GW], BF16, tag=f"npre{d}", name=f"npre_{d}{s}")
                    nc.vector.tensor_tensor(out=npre[d], in0=hn, in1=giN_s[d], op=Alu.add)
                    # ACT: tanh
                    n[d] = work.tile([128, GW], BF16, tag=f"n{d}", name=f"n_{d}{s}")
                    nc.scalar.activation(n[d], npre[d], AF.Tanh)

                # prefetch + pre-inject for step s+1 (PE slots after this
                # step's sigmoids have drained the RZ banks)
                if s + 1 < T:
                    inject_rz(s + 1)
                    fetch_gin(s + 1)

                # DVE phase 2: h' = n + z*(h-n); both dirs' h' land in one
                # paired tile so ymax runs as a single [128, 1024] op.
                hpair = work.tile([128, 2 * GW], BF16, tag="hpair", name=f"hp_{s}")
                for i, d in enumerate(DIRS):
                    dd = work.tile([128, GW], BF16, tag=f"d{d}", name=f"d_{d}{s}")
                    nc.vector.tensor_tensor(out=dd, in0=h[d], in1=n[d], op=Alu.subtract)
                    e = work.tile([128, GW], BF16, tag=f"e{d}", name=f"e_{d}{s}")
                    nc.vector.tensor_tensor(out=e, in0=rz[d][:, GW:2 * GW], in1=dd, op=Alu.mult)
                    nc.vector.tensor_tensor(out=hpair[:, i * GW:(i + 1) * GW], in0=n[d], in1=e, op=Alu.add)

                # running max-pool, off the critical path
                if s == 0:
                    nc.vector.tensor_copy(out=ymax, in_=hpair)
                else:
                    nc.vector.tensor_tensor(out=ymax, in0=ymax, in1=hpair, op=Alu.max)
                for i, d in enumerate(DIRS):
                    h[d] = hpair[:, i * GW:(i + 1) * GW]

            # output DMA split across queues to shorten the tail
            HGW = GW // 2
            for i, d in enumerate(DIRS):
                for k, eng in enumerate((nc.sync, nc.gpsimd, nc.scalar, nc.gpsimd)):
                    sl = slice(k * HGW // 2, (k + 1) * HGW // 2)
                    eng.dma_start(out=d_out[d][:, sl], in_=ymax[:, slice(i * GW + sl.start, i * GW + sl.stop)])

    nc.compile()
    return nc


def _prep_inputs(x, emb, Wih_f, Whh_f, bih_f, bhh_f, Wih_b, Whh_b, bih_b, bhh_b):
    """Host-side: projected-table gather of per-step gi tiles + weight packing."""
    import ml_dtypes

    f32 = np.float32
    bf16 = ml_dtypes.bfloat16
    x_flat = np.asarray(x).reshape(16384, T).astype(np.int32)

    embf = np.asarray(emb, f32)

    # P[dir][v, gate*32+dim] = emb[v] @ Wih[gate]^T + folded biases
    def proj_table(Wih, bih, bhh):
        Wih, bih, bhh = np.asarray(Wih, f32), np.asarray(bih, f32), np.asarray(bhh, f32)
        P = embf @ Wih.T  # [V, 96] (gates r,z,n)
        P[:, 0:H] += bih[0:H] + bhh[0:H]          # r: both biases
        P[:, H:2 * H] += bih[H:2 * H] + bhh[H:2 * H]  # z: both biases
        P[:, 2 * H:] += bih[2 * H:]               # n: ih bias only
        return P.astype(bf16)

    Ptab = {"f": proj_table(Wih_f, bih_f, bhh_f), "b": proj_table(Wih_b, bih_b, bhh_b)}

    # hh weight tiles: 4-block diag of Whh[gate].T (4 word-groups share K=128)
    def hh_tile(Whh, gate):
        L = np.zeros((128, 128), f32)
        Wg = np.asarray(Whh, f32)[gate * H:(gate + 1) * H, :]  # [32, 32]
        for g in range(NG):
            L[g * H:(g + 1) * H, g * H:(g + 1) * H] = Wg.T
        return L.astype(bf16)

    hh = {}
    for d, Whh in (("f", Whh_f), ("b", Whh_b)):
        for gi_, g in enumerate("RZN"):
            hh[(d, g)] = hh_tile(Whh, gi_)

    ident = np.eye(128, dtype=f32).astype(bf16)
    bhhN = {
        "f": np.ascontiguousarray(np.tile(np.asarray(bhh_f, f32)[2 * H:], NG).reshape(128, 1)),
        "b": np.ascontiguousarray(np.tile(np.asarray(bhh_b, f32)[2 * H:], NG).reshape(128, 1)),
    }

    in_maps = []
    for core in range(NCORES):
        xc = x_flat[core * WPC:(core + 1) * WPC]      # [2048, 16]
        xg = xc.reshape(NG, GW, T)                    # [4, 512, 16]
        m = {}
        for d in DIRS:
            # chars consumed at step s: forward -> s, backward -> T-1-s
            ch = xg if d == "f" else xg[:, :, ::-1]
            # P[ch] -> [4, 512, 16, 96] ; want per step [96? -> gates] tiles
            gathered = Ptab[d][ch]                    # [4, 512, 16, 96] bf16
            # tile[p = 32*g + dim, w] per (step, gate)
            # -> transpose to [16, 96, 4, 512] then split gates
            gt = np.ascontiguousarray(gathered.transpose(2, 3, 0, 1))  # [16,96,4,512]
            gt = gt.reshape(T, 3, H, NG, GW).transpose(0, 1, 3, 2, 4)  # [16,3,4,32,512]
            gt = gt.reshape(T, 3, 128, GW)
            m[f"giRZ_{d}"] = np.ascontiguousarray(
                np.concatenate([gt[:, 0], gt[:, 1]], axis=2)  # [16,128,1024]
            )
            m[f"giN_{d}"] = np.ascontiguousarray(gt[:, 2])    # [16,128,512]
            for g in "RZN":
                m[f"hh{g}_{d}"] = hh[(d, g)]
            m[f"bhhN_{d}"] = bhhN[d]
        m["ident"] = ident
        in_maps.append(m)
    return in_maps


def _install_ntff_hook():
    """Register the axon NTFF profiling hook (the image's antenv lacks
    axon_hooks, so run_bass_kernel_spmd's trace path can't find it)."""
    import types
    import antenv

    if "antenv.axon_hooks" in sys.modules:
        return
    mod = types.ModuleType("antenv.axon_hooks")
    _h = {"hook": None}
    mod.set_axon_ntff_profile_hook = lambda h: _h.update(hook=h)
    mod.get_axon_ntff_profile_hook = lambda: _h["hook"]
    sys.modules["antenv.axon_hooks"] = mod
    antenv.axon_hooks = mod
    try:
        from trn_agent_boot.trn_boot import _ntff_profile_via_ctypes

        hook = _ntff_profile_via_ctypes("/opt/axon/libaxon_pjrt.so")
        if hook is not None:
            mod.set_axon_ntff_profile_hook(hook)
    except Exception as e:  # profiling is best-effort
        print("ntff hook install failed:", e)
    # artifact upload needs a bucket that doesn't exist in this sandbox
    import concourse.bass_utils as bu

    bu.upload_artifacts = lambda tmpdir: tmpdir


def kernel(x, emb, Wih_f, Whh_f, bih_f, bhh_f, Wih_b, Whh_b, bih_b, bhh_b):
    if "nc" not in _CACHE:
        _CACHE["nc"] = _build_program()
    nc = _CACHE["nc"]

    in_maps = _prep_inputs(
        x, emb, Wih_f, Whh_f, bih_f, bhh_f, Wih_b, Whh_b, bih_b, bhh_b
    )

    trace = bool(int(os.environ.get("CHAR_RNN_TRACE", "0")))
    if trace:
        _install_ntff_hook()
    res = run_bass_kernel_spmd(
        nc, in_maps, core_ids=list(range(NCORES)), trace=trace,
        trace_cores=[0] if trace else None,
    )
    _CACHE["last_results"] = res

    out = np.empty((16384, 2 * H), np.float32)
    for core in range(NCORES):
        base = core * WPC
        for col, d in ((0, "f"), (H, "b")):
            o = res.results[core][f"out_{d}"].astype(np.float32)  # [128, 512]
            # o[32*g + dim, w] -> out[base + 512*g + w, dim]
            o = o.reshape(NG, H, GW).transpose(0, 2, 1).reshape(WPC, H)
            out[base:base + WPC, col:col + H] = o
    return out.reshape(B, S, 2 * H)
